# revision 1
# baseline (speedup 1.0000x reference)
"""Bilateral-solver-3D loss kernel for 8 TRN2 NeuronCores.

Loss = n_pix*LAM*mean(w_ij * d^2) + mean((output-target)^2), where
d[k,t,h,w] = output[t,h,w] - xp[t+kt, h+i, w+j] over K=2204 offsets
(kt,i,j) of a 5x21x21 stencil (center removed), xp = edge-padded output.

Strategy (memory-bound: the 282MB w_ij stream dominates):
  - Shard spatially: core c owns h in [10c, 10c+10) for all t -> 50
    (t,h) pairs per core; every core sees all K offsets. SPMD-uniform
    program; only the data differs per core.
  - On-chip layout: partition axis = w (80 lanes), free axis = padded
    offset index kidx = i*110 + j*5 + kt (KPAD = 21*22*5 = 2310; the
    center and the j=21 columns carry w=0 so they contribute nothing).
  - Per (t,h) pair: ScalarE computes d2 = Square(-xs + x) in ONE op,
    reading xs directly as a strided window view of a per-partition
    sliding-window tensor xps[w, (j,tp,hp)] = xp[tp, hp, w+j] (host
    prepared, bf16) with per-partition bias x[t,h,w]. VectorE does one
    bf16 2x tensor_tensor multiply m2 = w * d2. TensorE contracts the
    partition axis with a ones column into PSUM, accumulating across
    all 50 pairs. Tiny final reduce + scale on device; the host adds
    the 8 per-core scalars.
  - w is quantized to bf16 on the host (sum rel-err ~1e-6, way inside
    tolerance) halving HBM traffic.
"""

import os
import sys

import numpy as np

_TRN_REPO = "/opt/trn_rl_repo"
if _TRN_REPO not in sys.path:
    sys.path.insert(0, _TRN_REPO)

# ---- problem geometry (hardcoded per contract) ----
T, H, W = 5, 80, 80
TK, SK = 5, 21
CT, CS = 2, 10
LAM = 128.0
KTRUE = 2204
NI, NJ, NKT = 21, 22, 5          # i window, j window (incl. dead j=21), kt
KBLK = NI * NJ                   # 462 = one kt-plane of offsets
KPAD = KBLK * NKT                # 2310, kidx = kt*462 + i*22 + j
NCORES = 8
HB = H // NCORES                 # 10 h-rows per core
PAIRS = T * HB                   # 50 (t, h_local) pairs per core
TP = T + 2 * CT                  # 9  padded T
HPW = HB + 2 * CS                # 30 padded-h window height per core
WP1 = W + 2 * CS + 1             # 101 padded W (+1 col for the j=21 reads)
XPS_FREE = NJ * TP * HPW         # 5940 elements per partition
GP = 10                          # pairs per w-DMA chunk
NB = 5                           # PSUM bank chunks of KPAD
KB = KPAD // NB                  # 462
N_PIX = T * H * W                # 32000
FID_P, FID_F = 128, N_PIX // 128  # fidelity tile (128, 250)

LAST_RESULTS = None  # BassKernelResults of the most recent run (for test.py)

_CACHE = {}


def _offsets():
    offs = [
        (k, i, j)
        for i in range(SK)
        for j in range(SK)
        for k in range(TK)
        if not (i == CS and j == CS and k == CT)
    ]
    assert len(offs) == KTRUE
    return offs


def _build_nc():
    import concourse.bass as bass
    import concourse.mybir as mybir
    import concourse.tile as tile

    # -- walrus workaround: this container's walrus rejects any instruction
    # carrying >1 sync-wait and any drain resetting a multi-sem range
    # ("Too many sync wait commands"). Chunk resets; split waits onto
    # single-wait NOPs inserted before the instruction.
    def _chunked_dma_reset(self, semaphore_range=None):
        if semaphore_range is None:
            semaphore_range = self.bass._kernel_sem_range
        out = None
        for s in list(semaphore_range):
            out = self.drain(semaphore_range=range(s, s + 1))
        return out

    bass.BassGpSimd.dma_reset = _chunked_dma_reset

    def _split_multi_waits(nc):
        n_split = 0
        for f in nc.m.functions:
            for bb in f.blocks:
                insts = list(bb.instructions)
                out = []
                changed = False
                for ins in insts:
                    si = ins.sync_info
                    if si is not None and len(si.on_wait) > 1:
                        waits = list(si.on_wait)
                        for wi, wct in enumerate(waits[:-1]):
                            nop = mybir.InstNoOp(
                                name=f"{ins.name}-w{wi}",
                                sync_info=mybir.SyncInfo(
                                    on_wait=[wct], on_update=[]
                                ),
                                bass_nofuse=True,
                                engine=ins.engine,
                            )
                            nc.register_instruction(nop, overwrite=True)
                            out.append(nop)
                        ins.sync_info = mybir.SyncInfo(
                            on_wait=[waits[-1]], on_update=list(si.on_update)
                        )
                        changed = True
                        n_split += 1
                    out.append(ins)
                if changed:
                    bb.instructions = out
        return n_split

    bf16 = mybir.dt.bfloat16
    f32 = mybir.dt.float32

    nc = bass.Bass()
    w_d = nc.dram_tensor("w", [W, PAIRS * KPAD], bf16, kind="ExternalInput")
    xps_d = nc.dram_tensor("xps", [W, XPS_FREE], bf16, kind="ExternalInput")
    xc_d = nc.dram_tensor("xc", [W, PAIRS], f32, kind="ExternalInput")
    xf_d = nc.dram_tensor("xf", [FID_P, FID_F], f32, kind="ExternalInput")
    tf_d = nc.dram_tensor("tf", [FID_P, FID_F], f32, kind="ExternalInput")
    out_d = nc.dram_tensor("out", [1, 1], f32, kind="ExternalOutput")

    def win_view(ap, dims, extra_off):
        """Custom strided (overlapping) view of an SBUF tile AP."""
        v = ap.copy()
        p0 = v.ap[0]
        v.ap = mybir.VecI64Pair([list(p0)] + [list(d) for d in dims])
        v.offset = v.offset + extra_off
        return v

    with tile.TileContext(nc) as tc:
        with (
            tc.tile_pool(name="const", bufs=1) as cpool,
            tc.tile_pool(name="wbuf", bufs=2) as wpool,
            tc.tile_pool(name="d2buf", bufs=3) as d2pool,
            tc.tile_pool(name="m2buf", bufs=3) as m2pool,
            tc.tile_pool(name="d4buf", bufs=3) as d4pool,
            tc.tile_pool(name="psum", bufs=1, space="PSUM") as psum_pool,
        ):
            xps = cpool.tile([W, XPS_FREE], bf16)
            nc.sync.dma_start(xps[:], xps_d[:])
            xc = cpool.tile([W, PAIRS], f32)
            nc.sync.dma_start(xc[:], xc_d[:])
            ones80 = cpool.tile([W, 1], bf16)
            nc.vector.memset(ones80[:], 1.0)

            ps = psum_pool.tile([1, NB, 512], f32)

            # xps free layout: (tp, hp, j) -> elem = tp*660 + hp*22 + j.
            # ACT handles kt-planes 0..3 (plus all 5 for "full" pairs);
            # DVE handles kt=4 on split pairs (41 of 50) to balance engines.
            for g in range(PAIRS // GP):
                wt = wpool.tile([W, GP * KPAD], bf16)
                nc.sync.dma_start(
                    wt[:], w_d[:, g * GP * KPAD : (g + 1) * GP * KPAD]
                )
                for pl in range(GP):
                    p = g * GP + pl
                    t, hl = p // HB, p % HB
                    full_act = (p % 6 == 3) or p == 49
                    off = t * HPW * NJ + hl * NJ
                    d2 = d2pool.tile([W, KPAD], bf16)
                    nkt_act = NKT if full_act else NKT - 1
                    xs = win_view(
                        xps[:],
                        [[HPW * NJ, nkt_act], [NJ, NI], [1, NJ]],
                        off,
                    )
                    nc.scalar.activation(
                        d2[:, 0 : nkt_act * KBLK],
                        xs,
                        mybir.ActivationFunctionType.Square,
                        bias=xc[:, p : p + 1],
                        scale=-1.0,
                    )
                    if not full_act:
                        xs4 = win_view(
                            xps[:],
                            [[NJ, NI], [1, NJ]],
                            off + (NKT - 1) * HPW * NJ,
                        )
                        d4 = d4pool.tile([W, KBLK], bf16)
                        nc.vector.tensor_scalar(
                            d4[:],
                            xs4,
                            -1.0,
                            xc[:, p : p + 1],
                            op0=mybir.AluOpType.mult,
                            op1=mybir.AluOpType.add,
                        )
                        nc.vector.tensor_tensor(
                            d2[:, (NKT - 1) * KBLK : KPAD],
                            d4[:],
                            d4[:],
                            op=mybir.AluOpType.mult,
                        )
                    m2 = m2pool.tile([W, KPAD], bf16)
                    nc.vector.tensor_tensor(
                        m2[:],
                        wt[:, pl * KPAD : (pl + 1) * KPAD],
                        d2[:],
                        op=mybir.AluOpType.mult,
                    )
                    for b in range(NB):
                        nc.tensor.matmul(
                            ps[0:1, b, 0:KB],
                            ones80[:],
                            m2[:, b * KB : (b + 1) * KB],
                            start=(p == 0),
                            stop=(p == PAIRS - 1),
                        )

            # ---- final reduction of the smooth term ----
            s5 = cpool.tile([1, NB, KB], f32)
            nc.vector.tensor_copy(s5[:], ps[0:1, :, 0:KB])
            stot = cpool.tile([1, 1], f32)
            nc.vector.reduce_sum(stot[:], s5[:], axis=mybir.AxisListType.XY)

            # ---- fidelity term (identical on every core; host sums /8) ----
            fx = cpool.tile([FID_P, FID_F], f32)
            nc.sync.dma_start(fx[:], xf_d[:])
            ft = cpool.tile([FID_P, FID_F], f32)
            nc.sync.dma_start(ft[:], tf_d[:])
            fd = cpool.tile([FID_P, FID_F], f32)
            nc.vector.tensor_tensor(
                fd[:], fx[:], ft[:], op=mybir.AluOpType.subtract
            )
            fsq = cpool.tile([FID_P, FID_F], f32)
            nc.scalar.square(fsq[:], fd[:])
            frow = cpool.tile([FID_P, 1], f32)
            nc.vector.reduce_sum(frow[:], fsq[:], axis=mybir.AxisListType.X)
            ones128 = cpool.tile([FID_P, 1], f32)
            nc.vector.memset(ones128[:], 1.0)
            psf = psum_pool.tile([1, 1], f32)
            nc.tensor.matmul(psf[:], ones128[:], frow[:], start=True, stop=True)

            # ---- combine: out = stot*LAM/KTRUE + fid/(NCORES*n_pix) ----
            r1 = cpool.tile([1, 1], f32)
            nc.vector.tensor_scalar_mul(r1[:], stot[:], LAM / KTRUE)
            r2 = cpool.tile([1, 1], f32)
            nc.vector.tensor_scalar_mul(r2[:], psf[:], 1.0 / (NCORES * N_PIX))
            res = cpool.tile([1, 1], f32)
            nc.vector.tensor_tensor(
                res[:], r1[:], r2[:], op=mybir.AluOpType.add
            )
            nc.sync.dma_start(out_d[:], res[:])

    _split_multi_waits(nc)
    return nc


def _prep_inputs(w_ij, target, output):
    import ml_dtypes

    bf16 = ml_dtypes.bfloat16
    x = np.ascontiguousarray(output, dtype=np.float32)
    tgt = np.ascontiguousarray(target, dtype=np.float32)

    # padded volume with one extra w column for the dead j=21 reads
    xp = np.pad(x, ((CT, CT), (CS, CS), (CS, CS)), mode="edge")
    xp101 = np.concatenate([xp, xp[:, :, -1:]], axis=2)  # (9, 100, 101)
    xpb = xp101.astype(bf16)

    # sliding window over w+j: sw[tp, hp, w, j] = xpb[tp, hp, w+j]
    sw = np.lib.stride_tricks.sliding_window_view(xpb, NJ, axis=2)
    assert sw.shape == (TP, 2 * CS + H, W, NJ)

    xb3 = x.astype(bf16).astype(np.float32)  # (T, H, W) rounded like xps

    # w reorder: arr[w, t, h, n] then scatter n -> kidx
    offs = _offsets()
    kidx = np.array([k * KBLK + i * NJ + j for (k, i, j) in offs])
    arr = np.ascontiguousarray(
        np.asarray(w_ij, dtype=np.float32).transpose(3, 1, 2, 0)
    ).astype(bf16)  # (W, T, H, KTRUE)

    xf = x.reshape(FID_P, FID_F)
    tf = tgt.reshape(FID_P, FID_F)

    in_maps = []
    for c in range(NCORES):
        h0 = HB * c
        w_re = np.zeros((W, T, HB, KPAD), dtype=bf16)
        w_re[:, :, :, kidx] = arr[:, :, h0 : h0 + HB, :]
        xps_c = np.ascontiguousarray(
            sw[:, h0 : h0 + HPW, :, :].transpose(2, 0, 1, 3)
        )  # (W, TP, HPW, NJ)
        xc_c = np.ascontiguousarray(
            xb3[:, h0 : h0 + HB, :].transpose(2, 0, 1)
        )  # (W, T, HB)
        in_maps.append(
            {
                "w": w_re.reshape(W, PAIRS * KPAD),
                "xps": xps_c.reshape(W, XPS_FREE),
                "xc": xc_c.reshape(W, PAIRS),
                "xf": xf,
                "tf": tf,
            }
        )
    return in_maps


def kernel(w_ij, target, output):
    global LAST_RESULTS
    from concourse.bass_utils import run_bass_kernel_spmd

    if "nc" not in _CACHE:
        _CACHE["nc"] = _build_nc()
    nc = _CACHE["nc"]

    in_maps = _prep_inputs(w_ij, target, output)
    r = run_bass_kernel_spmd(nc, in_maps, core_ids=list(range(NCORES)))
    LAST_RESULTS = r
    total = np.float32(0.0)
    for c in range(NCORES):
        total = total + np.float32(r.results[c]["out"][0, 0])
    return np.asarray(total, dtype=np.float32)



# revision 41
# speedup vs baseline: 2.5753x; 2.5753x over previous
"""Bilateral-solver-3D loss kernel for 8 TRN2 NeuronCores (v4, 128-lane
+ clamp/mirror-folded half-stencil weights).

Loss = n_pix*LAM*mean(w_ij * d^2) + mean((output-target)^2), where
d[k,t,h,w] = output[t,h,w] - xp[t+kt, h+i, w+j] over K=2204 offsets
(kt,i,j) of a 5x21x21 stencil (center removed), xp = edge-padded output.

Key host-side identity (halves DMA and compute): edge padding is
clamping, so every term is w*(x[p]-x[c])^2 with c = clamp(p+off) a real
pixel. Re-binning each weight onto its *effective* offset d = c-p
(separable per axis), then folding mirror pairs (d,p) <-> (-d,p+d)
(always in-bounds after re-binning), yields an equivalent weight tensor
W2 supported on the 1102 lex-positive offsets only:
  S = sum_p sum_{d>0} W2[d,p]*(x[p]-x[p+d])^2.

Device strategy:
  - Shard spatially: core c owns h in [10c,10c+10) -> 50 rows x 80
    w-cols = 4000 column-tasks; grouped into 250 runs of 16 consecutive
    w-cols; lane l in [0,125) holds runs (2l, 2l+1) -> 32 slots/lane.
  - Folded offsets live at kidx in (1102, 2205); the device streams the
    contiguous tail [1092, 2205) = 1113 entries/slot (11 dead, zeroed),
    padded to a 1114 stride for 4B-aligned slot starts.
  - A slot's 1113 stencil reads form a 2-dim AP [[36,53],[1,21]] into
    the per-lane sliding window xps (rows 52..104 of the 5x21x36 run
    window; kt-plane stride 756 = 21*36 factors through i).
  - Per slot: ACT computes d2 = Square(-xs+x) for the first QSPLIT of
    the 53 window rows, DVE the rest (tensor_scalar + square), then one
    bf16 2x tensor_tensor m2 = W2 * d2; TensorE contracts the partition
    axis with a ones column into 3 PSUM banks accumulating over slots.
  - Side inputs ride the idle GpSimd (SWDGE) DMA queue; the w2 stream
    owns the sync queue. W2 is bf16 (sum rel-err ~1e-5).
"""

import sys

import numpy as np

_TRN_REPO = "/opt/trn_rl_repo"
if _TRN_REPO not in sys.path:
    sys.path.insert(0, _TRN_REPO)

# ---- problem geometry (hardcoded per contract) ----
T, H, W = 5, 80, 80
TK, SK = 5, 21
CT, CS = 2, 10
LAM = 128.0
KTRUE = 2204
NCORES = 8
HB = H // NCORES                # 10 h-rows per core

KBLK = SK * SK                  # 441
KGRID = TK * KBLK               # 2205 full stencil grid
CENTER = CT * KBLK + CS * SK + CS  # 1102
QROW0 = CENTER // SK            # 52: first window row the device reads
KLO = QROW0 * SK                # 1092: first kidx streamed
KPF = KGRID - KLO               # 1113 streamed entries per slot
KPFS = KPF + 1                  # 1114 per-slot stride in w2 DRAM (4B align)
NQ = TK * SK - QROW0            # 53 window rows per slot
RUN = 16                        # consecutive w-cols per run
NRUNL = 2                       # runs per lane
LANES = 128
RLANES = 125
SLOTS = NRUNL * RUN             # 32
WROW = RUN + SK - 1             # 36 window row width
WIN = NQ * WROW                 # 1908 window elems per run
GP = 4                          # slots per w-DMA chunk
NB = 3                          # PSUM bank chunks: 3 x 371
KB3 = KPF // NB                 # 371
N_PIX = T * H * W
FID_P, FID_F = 128, N_PIX // 128

QSPLIT = 46                     # window rows computed by ACT (even; rest DVE)
FP8_TAIL = False                # stream w2 chunks 2+ as fp8e4 via SWDGE cast
WSCALE = 4.0                    # weight pre-scale (dodges fp8 subnormal floor)
HEAD_SLOTS = 12                 # slots DMAed as bf16 on the sync queue

LAST_RESULTS = None  # BassKernelResults of the most recent run (for test.py)

_CACHE = {}


def _offsets():
    offs = [
        (k, i, j)
        for i in range(SK)
        for j in range(SK)
        for k in range(TK)
        if not (i == CS and j == CS and k == CT)
    ]
    assert len(offs) == KTRUE
    return offs


def fold_weights(w_ij):
    """(2204,T,H,W) -> (KGRID,T,H,W) f32 supported on kidx > CENTER.

    Folds clamped (out-of-bounds) offsets onto their effective offset,
    then mirror pairs (d,p)<->(-d,p+d) onto the lex-positive half.
    """
    offs = _offsets()
    kidx = np.array([k * KBLK + i * SK + j for (k, i, j) in offs])
    Wg = np.zeros((KGRID, T, H, W), dtype=np.float32)
    Wg[kidx] = np.asarray(w_ij, dtype=np.float32)

    # t-axis clamp fold
    Wv = Wg.reshape(TK, KBLK, T, H, W)
    Wt = np.zeros_like(Wv)
    for kt in range(TK):
        for t in range(T):
            kte = int(np.clip(t + kt - CT, 0, T - 1) - t + CT)
            Wt[kte, :, t] += Wv[kt, :, t]

    # h-axis clamp fold
    Wv = Wt.reshape(TK, SK, SK, T, H, W)
    Wh = np.zeros_like(Wv)
    harr = np.arange(H)
    for i in range(SK):
        ie = np.clip(harr + i - CS, 0, H - 1) - harr + CS
        for val in np.unique(ie):
            hs = np.nonzero(ie == val)[0]
            Wh[:, int(val), :, :, hs.min() : hs.max() + 1, :] += Wv[
                :, i, :, :, hs.min() : hs.max() + 1, :
            ]

    # w-axis clamp fold
    Ww = np.zeros_like(Wh)
    warr = np.arange(W)
    for j in range(SK):
        je = np.clip(warr + j - CS, 0, W - 1) - warr + CS
        for val in np.unique(je):
            ws = np.nonzero(je == val)[0]
            Ww[:, :, int(val), :, :, ws.min() : ws.max() + 1] += Wh[
                :, :, j, :, :, ws.min() : ws.max() + 1
            ]

    We = Ww.reshape(KGRID, T, H, W)

    # mirror fold onto the lex-positive half (kidx_m = 2*CENTER - kidx)
    W2 = np.zeros_like(We)
    W2[CENTER + 1 :] = We[CENTER + 1 :]
    for kid in range(CENTER + 1, KGRID):
        kt, rem = divmod(kid, KBLK)
        i, j = divmod(rem, SK)
        src = We[2 * CENTER - kid]
        dst_sl, src_sl = [], []
        for d, L in ((kt - CT, T), (i - CS, H), (j - CS, W)):
            a0, a1 = max(0, -d), min(L, L - d)
            dst_sl.append(slice(a0, a1))
            src_sl.append(slice(a0 + d, a1 + d))
        W2[kid][tuple(dst_sl)] += src[tuple(src_sl)]
    return W2


def _build_nc():
    import concourse.bass as bass
    import concourse.mybir as mybir
    import concourse.tile as tile

    # -- walrus workaround: this container's walrus rejects any instruction
    # carrying >1 sync-wait and any drain resetting a multi-sem range
    # ("Too many sync wait commands"). Chunk resets; split waits onto
    # single-wait NOPs inserted before the instruction.
    def _chunked_dma_reset(self, semaphore_range=None):
        if semaphore_range is None:
            semaphore_range = self.bass._kernel_sem_range
        out = None
        for s in list(semaphore_range):
            out = self.drain(semaphore_range=range(s, s + 1))
        return out

    bass.BassGpSimd.dma_reset = _chunked_dma_reset

    def _split_multi_waits(nc):
        n_split = 0
        for f in nc.m.functions:
            for bb in f.blocks:
                insts = list(bb.instructions)
                out = []
                changed = False
                for ins in insts:
                    si = ins.sync_info
                    if si is not None and len(si.on_wait) > 1:
                        waits = list(si.on_wait)
                        for wi, wct in enumerate(waits[:-1]):
                            nop = mybir.InstNoOp(
                                name=f"{ins.name}-w{wi}",
                                sync_info=mybir.SyncInfo(
                                    on_wait=[wct], on_update=[]
                                ),
                                bass_nofuse=True,
                                engine=ins.engine,
                            )
                            nc.register_instruction(nop, overwrite=True)
                            out.append(nop)
                        ins.sync_info = mybir.SyncInfo(
                            on_wait=[waits[-1]], on_update=list(si.on_update)
                        )
                        changed = True
                        n_split += 1
                    out.append(ins)
                if changed:
                    bb.instructions = out
        return n_split

    bf16 = mybir.dt.bfloat16
    f32 = mybir.dt.float32
    fp8 = mybir.dt.float8e4

    nc = bass.Bass()
    nw = HEAD_SLOTS if FP8_TAIL else SLOTS
    w_d = nc.dram_tensor("w", [LANES, nw * KPFS], bf16, kind="ExternalInput")
    if FP8_TAIL:
        w8_d = nc.dram_tensor(
            "w8", [LANES, (SLOTS - HEAD_SLOTS) * KPFS], fp8, kind="ExternalInput"
        )
    xps_d = nc.dram_tensor("xps", [LANES, NRUNL * WIN], bf16, kind="ExternalInput")
    xc_d = nc.dram_tensor("xc", [LANES, SLOTS], f32, kind="ExternalInput")
    xf_d = nc.dram_tensor("xf", [FID_P, FID_F], f32, kind="ExternalInput")
    tf_d = nc.dram_tensor("tf", [FID_P, FID_F], f32, kind="ExternalInput")
    out_d = nc.dram_tensor("out", [1, 1], f32, kind="ExternalOutput")

    def win_view(ap, dims, extra_off):
        """Custom strided (overlapping) view of an SBUF tile AP."""
        v = ap.copy()
        p0 = v.ap[0]
        v.ap = mybir.VecI64Pair([list(p0)] + [list(d) for d in dims])
        v.offset = v.offset + extra_off
        return v

    with tile.TileContext(nc) as tc:
        with (
            tc.tile_pool(name="const", bufs=1) as cpool,
            tc.tile_pool(name="d2buf", bufs=8) as d2pool,
            tc.tile_pool(name="m2buf", bufs=6) as m2pool,
            tc.tile_pool(name="d4buf", bufs=3) as d4pool,
            tc.tile_pool(name="psum", bufs=1, space="PSUM") as psum_pool,
        ):
            # All inputs ride the sync (HWDGE) queue, interleaved so the
            # critical-path ones land first: xc, xps half 0, w chunk 0,
            # xps half 1, fidelity, then the w stream.
            xc = cpool.tile([LANES, SLOTS], f32)
            nc.sync.dma_start(xc[:], xc_d[:])
            xps = cpool.tile([LANES, NRUNL * WIN], bf16)
            nc.sync.dma_start(xps[:, 0:WIN], xps_d[:, 0:WIN])

            ones = cpool.tile([LANES, 2], bf16)  # padded to 4B for alignment
            nc.vector.memset(ones[:], 1.0)

            ps = psum_pool.tile([1, NB, 512], f32)

            nact = QSPLIT * SK              # ACT-computed prefix of d2
            ndve = KPF - nact               # DVE-computed suffix

            # Whole w tensor resident in SBUF (71.3 KB/lane): every DMA
            # slice issues upfront in data-need order, so the stream runs
            # at full HBM rate with no pool-recycle gating.
            wbig = cpool.tile([LANES, SLOTS * KPFS], bf16)

            def wdma(a, b):
                nc.sync.dma_start(
                    wbig[:, a * KPFS : b * KPFS], w_d[:, a * KPFS : b * KPFS]
                )

            wdma(0, 1)
            wdma(1, 2)
            wdma(2, 4)
            wdma(4, 8)
            wdma(8, 12)
            nc.sync.dma_start(xps[:, WIN : 2 * WIN], xps_d[:, WIN : 2 * WIN])
            wdma(12, 16)
            fx = cpool.tile([FID_P, FID_F], f32)
            nc.sync.dma_start(fx[:], xf_d[:])
            ft = cpool.tile([FID_P, FID_F], f32)
            nc.sync.dma_start(ft[:], tf_d[:])
            for a in range(16, SLOTS, GP):
                wdma(a, a + GP)

            def emit_mult(sp, d2p):
                # deferred one slot behind the d2 producers: keeps a stalled
                # (w-waiting) TTm2 from head-of-line-blocking the DVE queue
                m2 = m2pool.tile([LANES, KPF + 1], bf16)
                nc.vector.tensor_tensor(
                    m2[:, 0:KPF],
                    wbig[:, sp * KPFS : sp * KPFS + KPF],
                    d2p[:, 0:KPF],
                    op=mybir.AluOpType.mult,
                )
                for b in range(NB):
                    nc.tensor.matmul(
                        ps[0:1, b, 0:KB3],
                        ones[:, 0:1],
                        m2[:, b * KB3 : (b + 1) * KB3],
                        start=(sp == 0),
                        stop=(sp == SLOTS - 1),
                    )

            prev = None
            for s in range(SLOTS):
                if s == 20:
                    # fidelity compute once its inputs have landed
                    fd = cpool.tile([FID_P, FID_F], f32)
                    nc.vector.tensor_tensor(
                        fd[:], fx[:], ft[:], op=mybir.AluOpType.subtract
                    )
                    fsq = cpool.tile([FID_P, FID_F], f32)
                    nc.scalar.square(fsq[:], fd[:])
                    frow = cpool.tile([FID_P, 1], f32)
                    nc.vector.reduce_sum(
                        frow[:], fsq[:], axis=mybir.AxisListType.X
                    )
                    ones128 = cpool.tile([FID_P, 1], f32)
                    nc.vector.memset(ones128[:], 1.0)
                    psf = psum_pool.tile([1, 1], f32)
                    nc.tensor.matmul(
                        psf[:], ones128[:], frow[:], start=True, stop=True
                    )
                if True:
                    run_sel, r = s // RUN, s % RUN
                    woff = run_sel * WIN + r
                    # tiles padded to even free sizes: odd bf16 sizes would
                    # 2-byte-misalign later pool buffers and knock TT off 2x
                    d2 = d2pool.tile([LANES, KPF + 1], bf16)
                    nc.scalar.activation(
                        d2[:, 0:nact],
                        win_view(xps[:], [[WROW, QSPLIT], [1, SK]], woff),
                        mybir.ActivationFunctionType.Square,
                        bias=xc[:, s : s + 1],
                        scale=-1.0,
                    )
                    d4 = d4pool.tile([LANES, ndve + (ndve & 1)], bf16)
                    nc.vector.tensor_scalar(
                        d4[:, 0:ndve],
                        win_view(
                            xps[:],
                            [[WROW, NQ - QSPLIT], [1, SK]],
                            woff + WROW * QSPLIT,
                        ),
                        -1.0,
                        xc[:, s : s + 1],
                        op0=mybir.AluOpType.mult,
                        op1=mybir.AluOpType.add,
                    )
                    nc.vector.tensor_tensor(
                        d2[:, nact:KPF],
                        d4[:, 0:ndve],
                        d4[:, 0:ndve],
                        op=mybir.AluOpType.mult,
                    )
                    if prev is not None:
                        emit_mult(*prev)
                    prev = (s, d2)
            emit_mult(*prev)

            # ---- final reductions ----
            # PSUM -> scalar on ACT (idle at the tail; DVE still busy):
            # Identity activation with free-dim accumulate
            sred = cpool.tile([1, KPF + 1], f32)
            stot = cpool.tile([1, 1], f32)
            nc.scalar.activation(
                sred[:, 0:KPF],
                ps[0:1, :, 0:KB3],
                mybir.ActivationFunctionType.Identity,
                accum_out=stot[:],
            )
            r1 = cpool.tile([1, 1], f32)
            nc.vector.tensor_scalar_mul(r1[:], stot[:], LAM / KTRUE / WSCALE)
            r2 = cpool.tile([1, 1], f32)
            nc.vector.tensor_scalar_mul(r2[:], psf[:], 1.0 / (NCORES * N_PIX))
            res = cpool.tile([1, 1], f32)
            nc.vector.tensor_tensor(
                res[:], r1[:], r2[:], op=mybir.AluOpType.add
            )
            nc.sync.dma_start(out_d[:], res[:])

    _split_multi_waits(nc)

    # Prune exit-time per-sem DMA drains for semaphores no DMA updates —
    # each drain costs ~270ns of serial tail on the GpSimd queue.
    dma_sems = set()
    for f in nc.m.functions:
        for bb in f.blocks:
            for ins in bb.instructions:
                if isinstance(ins, mybir.InstDMACopy) and ins.sync_info:
                    for u in ins.sync_info.on_update:
                        dma_sems.add(u.id)
    for f in nc.m.functions:
        for bb in f.blocks:
            bb.instructions = [
                ins
                for ins in bb.instructions
                if not (
                    isinstance(ins, mybir.InstDrain)
                    and getattr(ins, "is_reset_sema", False)
                    and not any(
                        s in dma_sems
                        for s in range(
                            ins.reset_range_start, ins.reset_range_stop
                        )
                    )
                )
            ]
    return nc


def _prep_inputs(w_ij, target, output):
    import ml_dtypes

    bf16 = ml_dtypes.bfloat16
    x = np.ascontiguousarray(output, dtype=np.float32)
    tgt = np.ascontiguousarray(target, dtype=np.float32)

    xp = np.pad(x, ((CT, CT), (CS, CS), (CS, CS)), mode="edge")  # (9,100,100)
    xpb = xp.astype(bf16)
    # sw[a, b, c] = xpb[a:a+5, b:b+21, c:c+36] -> (5, 80, 65, 5, 21, 36)
    sw = np.lib.stride_tricks.sliding_window_view(xpb, (TK, SK, WROW))
    xb3 = x.astype(bf16).astype(np.float32)

    # folded half-stencil weights: (T, H, W, KPF), pre-scaled by WSCALE
    W2 = fold_weights(w_ij)[KLO:] * np.float32(WSCALE)  # (KPF, T, H, W)
    wT32 = np.ascontiguousarray(W2.transpose(1, 2, 3, 0))
    wT = wT32.astype(bf16)

    rows = np.arange(50)
    t_of_row, hl_of_row = rows // HB, rows % HB
    runs = np.arange(250)
    row_of_run, wblk_of_run = runs // 5, runs % 5
    t_arr = t_of_row[row_of_run]
    hl_arr = hl_of_row[row_of_run]
    w0_arr = RUN * wblk_of_run

    xf = x.reshape(FID_P, FID_F)
    tf = tgt.reshape(FID_P, FID_F)

    fp8 = ml_dtypes.float8_e4m3fn
    in_maps = []
    for c in range(NCORES):
        h0 = HB * c
        # w2: (50, 80, KPF) -> (125, 2, 16, KPF) with KPFS padding
        wc = wT[:, h0 : h0 + HB, :, :].reshape(RLANES, NRUNL, RUN, KPF)
        w_re = np.zeros((LANES, NRUNL, RUN, KPFS), dtype=bf16)
        w_re[:RLANES, :, :, :KPF] = wc
        # xps: per-run windows, rows 52..104 of the (105, 36) run window
        xps_c = np.zeros((LANES, NRUNL, WIN), dtype=bf16)
        xps_c[:RLANES] = (
            sw[t_arr, h0 + hl_arr, w0_arr]
            .reshape(250, TK * SK, WROW)[:, QROW0:, :]
            .reshape(RLANES, NRUNL, WIN)
        )
        xc_c = np.zeros((LANES, SLOTS), dtype=np.float32)
        xc_c[:RLANES] = xb3[:, h0 : h0 + HB, :].reshape(250, RUN).reshape(
            RLANES, SLOTS
        )
        wflat = w_re.reshape(LANES, SLOTS * KPFS)
        im = {
            "xps": xps_c.reshape(LANES, NRUNL * WIN),
            "xc": xc_c,
            "xf": xf,
            "tf": tf,
        }
        if FP8_TAIL:
            hs = HEAD_SLOTS * KPFS
            im["w"] = np.ascontiguousarray(wflat[:, :hs])
            # fp8 tail quantized from the f32 weights (not via bf16)
            wq = wT32[:, h0 : h0 + HB, :, :].reshape(RLANES, NRUNL, RUN, KPF)
            w8 = np.zeros((LANES, NRUNL, RUN, KPFS), dtype=fp8)
            w8[:RLANES, :, :, :KPF] = wq.astype(fp8)
            im["w8"] = np.ascontiguousarray(
                w8.reshape(LANES, SLOTS * KPFS)[:, hs:]
            )
        else:
            im["w"] = wflat
        in_maps.append(im)
    return in_maps


def kernel(w_ij, target, output):
    global LAST_RESULTS
    from concourse.bass_utils import run_bass_kernel_spmd

    if "nc" not in _CACHE:
        _CACHE["nc"] = _build_nc()
    nc = _CACHE["nc"]

    in_maps = _prep_inputs(w_ij, target, output)
    r = run_bass_kernel_spmd(nc, in_maps, core_ids=list(range(NCORES)))
    LAST_RESULTS = r
    total = np.float32(0.0)
    for c in range(NCORES):
        total = total + np.float32(r.results[c]["out"][0, 0])
    return np.asarray(total, dtype=np.float32)
